# revision 39
# baseline (speedup 1.0000x reference)
"""Trainium2 Bass kernel for nn_ATTLayer (GNN message-passing attention).

Reference math:
    m_i = T @ W.T + b                        [B, D]
    m_j = edge @ W.T + b                     [E, B, D]
    e_ij[e,b] = sum_f m_i[b,f] * m_j[e,b,f]  [E, B]
    out = (e_ij / sum_e e_ij)[:, :, None] * edge

Algebraic refactor (never materializes m_j; one read of edge):
    e_ij[e,b] = sum_d edge[e,b,d] * u[b,d] + c[b]
      u  = T @ G + h     with  G = W.T @ W   [D, D]
      c  = T @ g + c0    with  g = W.T @ b   [D],  h = b @ W,  c0 = b.b
G/g/h/c0 are tiny and computed on the host in float64.

Sharding: B=4096 split across 8 cores (512 each); E and D stay whole so
e_j_sum needs no cross-core reduction -> fully data-parallel, no collectives.

Numerics: e_j_sum is a signed sum over E*D=16K products with ~200x
cancellation (min |S| = 0.13 vs e_ij rms 26), so the e_ij path needs the
f32 edge read (bf16 edge was measured at rel-err 0.91).  The OUT stream is
bf16 (rel rounding 2^-9 -> ~5e-3 vs the 2e-2 gate).

Structure (measured 204-206 us typical; run-to-run noise up to +-9%,
vs the 3-pass baseline's 218-223):
  - e_raw: ONE fused DVE op per e — scalar_tensor_tensor(prod=edge_e*u,
    accum_out=sum_d) + the hw accumulator read (~280+76 ns/e; DVE paces the
    kernel at ~44.3 us/b-tile).  NOTES: tensor_tensor_reduce compiles but
    CRASHES the device (NRT unrecoverable); STT on Pool is rejected by
    walrus; GPSIMD free-axis tensor_reduce does not exist.  Offloading
    chunks as GP-mul + DVE-seg-reduce has lower nominal DVE content but
    measured worse (230) — GP overload + DVE/GPSIMD shared-SBUF-port
    blocking.  DVE carries NO rescale work (moving its 4 broadcast-TT
    units, 5.7 us each with the inner-stride-0 penalty, off DVE was worth
    ~9 us of wall).
  - gate: ACT activation(Identity, bias=c, accum_out=esum) folds +c and the
    e-sum into one idle-engine op; DVE reciprocal; ACT wts mul.
  - rescale (out = w * edge -> bf16): per tile 5 GPSIMD broadcast-TT units
    (3.64 us each) + 3 ACT per-e units (7.7 us each); ACT units are issued
    FIRST and ALL out-DMA issues go after the tile's rescale compute, so
    the ACT ring is never head-of-line blocked by a DMA issue waiting on a
    GP unit.  Last b-tile: 4 GP + 2 ACT + 2 DVE-tensor_scalar units (DVE is
    free in the drain).
  - u/c: PE-only — ones-row outer-product preloads h/c0 into PSUM
    (start=True), the T@G matmul accumulates (start=False), ACT copies
    PSUM->SBUF.  Zero DVE preamble ops.
  - constants host-prepacked (h/c0/ones replicated) into ONE [128, 898]
    tensor -> a single contiguous DMA (broadcast-AP loads ran at 9 GB/s).
  - scrd bufs=4 is load-bearing (bufs=1 costs ~60 us in per-STT WAW sems);
    bufs=8 measured worse.
The in-stream bursts at ~410 GB/s when buffers allow; HBM floor for the
48.4 MB/core of traffic is ~120-135 us — further gains need the fused
e_raw pass off DVE, which no working instruction currently allows.
"""

import numpy as np

import concourse.bacc as bacc
import concourse.bass as bass
import concourse.mybir as mybir
import concourse.tile as tile
from concourse.bass import ts

N_CORES = 8
E = 128
B = 4096
D = 128
BL = B // N_CORES  # 512 batch rows per core
BT = 128           # b-tile size (SBUF partition dim)

F32 = mybir.dt.float32
BF16 = mybir.dt.bfloat16

# consts layout (columns in the packed [128, NCONST] tensor)
C_G = 0            # G [128, 128]
C_GCOL = D         # g [128, 1]
C_H = D + 1        # h replicated [128, 128]
C_C0 = 2 * D + 1   # c0 replicated [128, 1]
C_ONES = 2 * D + 2  # 1.0s [128, 128] (row 0 used as PE ones-lhsT)
C_TT = 3 * D + 2   # temporalT [128, BL]
NCONST = 3 * D + 2 + BL

# Module-level cache so repeated kernel() calls reuse the compiled executable.
_CACHE = {}


def build_bass(
    bl=BL,
    n_e=E,
    eh_sz=16,            # e-slice: edge SBUF tiles are [128, eh, D] (1 MiB)
    ebufs=14,
    obufs=6,
    sbufs=10,
    ec=16,               # e-chunk for the mul/reduce (prod) ops (must divide eh)
    rc=16,               # rescale unit (e per rescale op)
    mul_pat="d",  # per-ec-chunk engine for the prod mul (g=GP, d=DVE);
                         # the d-reduce is always DVE (only engine with free-
                         # axis tensor_reduce). "/" = per-b-tile patterns.
    res_pat="gggggaaa/gggggaaa/gggggaaa/ggggaass",  # per-rc-unit rescale:
                         # g=GP bcast TT, s=DVE per-e tensor_scalar (278ns/e
                         # vs 485ns/e for DVE bcast TT), a=ACT per-e,
                         # d=DVE bcast TT.  'a' units are issued FIRST so the
                         # ACT ring isn't head-of-line blocked by out-DMA
                         # sem-waits. Last tile avoids 'a' (7.7us/unit serial
                         # in the drain).
    ttr_mode="chunk",      # stt: fused per-e STT; chunk: DVE mul+seg-reduce per ec
    out_dt="bf16",
    out_eng="scalar",    # engine issuing out-DMAs (HWDGE: scalar|sync only)
    scheme="real",       # real | dmaonly | dmafree (roofline probes)
):
    """Build the per-core Bass program. Same program runs SPMD on all cores."""
    nbt = bl // BT
    nrc = n_e // rc
    odt = BF16 if out_dt == "bf16" else F32
    nc = bacc.Bacc("TRN2", target_bir_lowering=False, debug=False)

    consts_d = nc.declare_dram_parameter("consts", [D, NCONST], F32, isOutput=False)
    edge_d = nc.declare_dram_parameter("edge", [bl, n_e, D], F32, isOutput=False)
    out_d = nc.declare_dram_parameter("out", [bl, n_e, D], odt, isOutput=True)

    def eng(ch):
        return {"a": nc.scalar, "g": nc.gpsimd, "d": nc.vector}[ch]

    out_ring = {
        "scalar": nc.scalar,
        "sync": nc.sync,
        "gpsimd": nc.gpsimd,
        "tensor": nc.tensor,
    }[out_eng]

    with tile.TileContext(nc) as tc:
        with (
            tc.tile_pool(name="singles", bufs=1) as singles,
            tc.tile_pool(name="edges", bufs=ebufs) as edges,
            tc.tile_pool(name="small", bufs=sbufs) as small,
            tc.tile_pool(name="scrd", bufs=(2 if ttr_mode == "chunk" else 4)) as scrd,
            tc.tile_pool(name="scrg", bufs=2) as scrg,
            tc.tile_pool(name="outs", bufs=obufs) as outs,
            tc.tile_pool(name="psum", bufs=2, space="PSUM") as psum,
        ):
            # ---- constants: ONE contiguous DMA, first on the in ring ----
            cst = singles.tile([128, NCONST], F32, tag="consts")
            nc.sync.dma_start(out=cst, in_=consts_d[:])
            g_sb = cst[:, C_G:C_G + D]
            gc_sb = cst[:, C_GCOL:C_GCOL + 1]
            h_sb = cst[:, C_H:C_H + D]
            c0_sb = cst[:, C_C0:C_C0 + 1]
            tempT = cst[:, C_TT:C_TT + bl]

            # ---- edge prefetch ----
            eh = min(eh_sz, n_e)
            n_sl = (n_e + eh - 1) // eh

            def load_tile(i):
                slices = []
                for hf in range(n_sl):
                    et = edges.tile([128, eh, D], F32, tag="edge")
                    nc.sync.dma_start(
                        out=et, in_=edge_d[ts(i, BT), ts(hf, eh), :]
                    )
                    slices.append(et)
                return slices

            pending = [load_tile(0)]

            # ---- u = T @ G + h   and   c = T @ g + c0, per b-tile ----
            # +h / +c0 are folded into PE as a ones-row outer-product PSUM
            # preload (start=True) followed by the real matmul (start=False),
            # and the PSUM->SBUF move goes on idle ACT: zero DVE preamble ops.
            ones_row = cst[0:1, C_ONES:C_ONES + D]
            h_row = cst[0:1, C_H:C_H + D]
            c0_row = cst[0:1, C_C0:C_C0 + 1]
            u_t, c_t = [], []
            for i in range(nbt):
                u_ps = psum.tile([128, 128], F32, tag="u_ps")
                nc.tensor.matmul(u_ps, lhsT=ones_row, rhs=h_row,
                                 start=True, stop=False)
                nc.tensor.matmul(u_ps, lhsT=tempT[:, ts(i, BT)], rhs=g_sb,
                                 start=False, stop=True)
                u_i = singles.tile([128, 128], F32, tag=f"u_t{i}")
                nc.scalar.copy(u_i, u_ps)
                u_t.append(u_i)
                c_ps = psum.tile([128, 1], F32, tag="c_ps")
                nc.tensor.matmul(c_ps, lhsT=ones_row, rhs=c0_row,
                                 start=True, stop=False)
                nc.tensor.matmul(c_ps, lhsT=tempT[:, ts(i, BT)], rhs=gc_sb,
                                 start=False, stop=True)
                c_i = singles.tile([128, 1], F32, tag=f"c_t{i}")
                nc.scalar.copy(c_i, c_ps)
                c_t.append(c_i)

            scratch_t = None
            if scheme == "dmafree":
                scratch_t = singles.tile([128, 2 * rc, D], odt, tag="scr")
                nc.vector.memset(scratch_t[:, 0, :], 1.0)

            # ---- main loop over b-tiles ----
            for i in range(nbt):
                slices = pending.pop(0)
                if i + 1 < nbt:
                    pending.append(load_tile(i + 1))

                if scheme == "dmaonly":
                    for hf in range(n_sl):
                        ot = outs.tile([128, eh, D], odt, tag="out")
                        nc.vector.tensor_copy(ot, slices[hf])
                        out_ring.dma_start(
                            out=out_d[ts(i, BT), ts(hf, eh), :], in_=ot
                        )
                    continue
                if scheme == "dmafree":
                    for j in range(0, nrc, 2):
                        out_ring.dma_start(
                            out=out_d[ts(i, BT), j * rc : (j + 2) * rc, :],
                            in_=scratch_t,
                        )
                    continue

                # ---- e_raw[b, e] = sum_d edge[b, e, d] * u[b, d] ----
                # fused mul + d-reduce per e in ONE DVE op
                # (scalar_tensor_tensor + accum_out).  tensor_tensor_reduce
                # compiles but crashes the device; STT on Pool is rejected by
                # walrus, so this pass is DVE-only and GPSIMD takes most of
                # the rescale instead.
                u_i = u_t[i]
                u_ap = u_i[:, :]
                u_bcast = bass.AP(
                    tensor=u_ap.tensor,
                    offset=u_ap.offset,
                    ap=[u_ap.ap[0], [0, ec], u_ap.ap[1]],
                )
                mp = mul_pat.split("/")[min(i, len(mul_pat.split("/")) - 1)]
                e_raw = small.tile([128, n_e], F32, tag="e_raw")
                g_chunks = []
                for j in range(n_e // ec):
                    if ttr_mode == "chunk":
                        # all-DVE: clean middle-broadcast mul (2.74us/16e)
                        # + segmented reduce (2.29us/16e) = 314ns/e vs the
                        # per-e STT's 356ns/e, and ~10x fewer DVE ops.
                        scr = scrd.tile([128, ec, D], F32, tag="scrd")
                        nc.vector.tensor_mul(
                            scr, slices[j][:, ts(0, ec), :], u_bcast
                        )
                        nc.vector.tensor_reduce(
                            out=e_raw[:, ts(j, ec)],
                            in_=scr,
                            axis=mybir.AxisListType.X,
                            op=mybir.AluOpType.add,
                        )
                        continue
                    if mp[j % len(mp)] == "g":
                        # GP mul chunk + (deferred) DVE segmented reduce:
                        # 2.3us of DVE instead of ~5.9us of per-e STT.
                        scr = scrg.tile([128, ec, D], F32, tag="scrg")
                        nc.gpsimd.tensor_mul(
                            scr, slices[j][:, ts(0, ec), :], u_bcast
                        )
                        g_chunks.append((j, scr))
                        continue
                    for e in range(j * ec, (j + 1) * ec):
                        ch = slices[e // eh][:, e % eh, :]
                        scr = scrd.tile([128, 128], F32, tag="scrd")
                        nc.vector.scalar_tensor_tensor(
                            out=scr,
                            in0=ch,
                            scalar=1.0,
                            in1=u_i,
                            op0=mybir.AluOpType.mult,
                            op1=mybir.AluOpType.mult,
                            accum_out=e_raw[:, e:e + 1],
                        )
                for j, scr in g_chunks:
                    nc.vector.tensor_reduce(
                        out=e_raw[:, ts(j, ec)],
                        in_=scr,
                        axis=mybir.AxisListType.X,
                        op=mybir.AluOpType.add,
                    )

                # ---- gate: e_ij = e_raw + c (ACT, fused esum) ----
                e_ij = small.tile([128, n_e], F32, tag="e_ij")
                esum = small.tile([128, 1], F32, tag="esum")
                nc.scalar.activation(
                    out=e_ij,
                    in_=e_raw,
                    func=mybir.ActivationFunctionType.Identity,
                    bias=c_t[i],
                    scale=1.0,
                    accum_out=esum,
                )
                winv = small.tile([128, 1], F32, tag="winv")
                nc.vector.reciprocal(winv, esum)
                wts = small.tile([128, n_e], F32, tag="wts")
                nc.scalar.mul(wts, e_ij, winv)

                # ---- rescale: out[b, e, :] = wts[b, e] * edge[b, e, :] ----
                # 'a' (ACT per-e) units run FIRST and all out-DMA issues go
                # AFTER the tile's rescale compute on the ACT ring, so ACT's
                # own units are never head-of-line blocked by a DMA issue
                # waiting on a GP/DVE unit.
                rp = res_pat.split("/")[min(i, len(res_pat.split("/")) - 1)]
                otl = {}
                order = sorted(range(nrc), key=lambda j: rp[j % len(rp)] != "a")
                for j in order:
                    e0 = j * rc
                    et = slices[e0 // eh]
                    ch = et[:, e0 % eh : e0 % eh + rc, :]
                    if j // 2 not in otl:
                        ot = outs.tile([128, 2 * rc, D], odt, tag="out")
                        otl[j // 2] = ot
                    ot = otl[j // 2]
                    o_sl = ot[:, ts(j % 2, rc), :]
                    e_sel = rp[j % len(rp)]
                    if e_sel == "a":
                        for ee in range(rc):
                            nc.scalar.mul(
                                o_sl[:, ee, :],
                                et[:, e0 % eh + ee, :],
                                wts[:, e0 + ee : e0 + ee + 1],
                            )
                    elif e_sel == "s":
                        for ee in range(rc):
                            nc.vector.tensor_scalar(
                                out=o_sl[:, ee, :],
                                in0=et[:, e0 % eh + ee, :],
                                scalar1=wts[:, e0 + ee : e0 + ee + 1],
                                scalar2=None,
                                op0=mybir.AluOpType.mult,
                            )
                    else:
                        w_sl = wts[:, ts(j, rc)]
                        w_bcast = bass.AP(
                            tensor=w_sl.tensor,
                            offset=w_sl.offset,
                            ap=[w_sl.ap[0], w_sl.ap[1], [0, D]],
                        )
                        eng(e_sel).tensor_mul(o_sl, ch, w_bcast)
                for p in range(nrc // 2):
                    out_ring.dma_start(
                        out=out_d[ts(i, BT), 2 * p * rc : 2 * (p + 1) * rc, :],
                        in_=otl[p],
                    )
    nc.compile()
    return nc


def _host_precompute(W, b):
    W64 = W.astype(np.float64)
    b64 = b.astype(np.float64)
    G = (W64.T @ W64).astype(np.float32)
    g = (W64.T @ b64).astype(np.float32)
    h = (b64 @ W64).astype(np.float32)
    c0 = np.float32(b64 @ b64)
    return G, g, h, c0


def make_feed(T, edge, W, b):
    """Build the global-concat feed dict for shard_map's axis-0 split."""
    G, g, h, c0 = _host_precompute(W, b)
    consts = np.empty((N_CORES, D, NCONST), dtype=np.float32)
    consts[:, :, C_G:C_G + D] = G
    consts[:, :, C_GCOL] = g[None, :]  # g down the partition axis
    consts[:, :, C_H:C_H + D] = h[None, None, :]
    consts[:, :, C_C0] = c0
    consts[:, :, C_ONES:C_ONES + D] = 1.0
    TT = T.T.reshape(D, N_CORES, BL).transpose(1, 0, 2)  # per-core T^T blocks
    consts[:, :, C_TT:C_TT + BL] = TT
    return {
        "consts": np.ascontiguousarray(consts.reshape(N_CORES * D, NCONST)),
        "edge": np.ascontiguousarray(edge.transpose(1, 0, 2)),
    }


def _enable_neff_cache(bass2jax):
    """Cache walrus NEFF compiles in /tmp keyed on the BIR hash, so repeat
    kernel() invocations in fresh processes skip the ~20-60s compile."""
    if getattr(bass2jax, "_att_neff_cache", False):
        return
    import hashlib
    import os
    import re
    import shutil
    import tempfile

    orig = bass2jax.compile_bir_kernel
    cache_dir = "/tmp/att_neff_cache"

    def cached(bir_json, tmpdir, neff_name="file.neff"):
        try:
            os.makedirs(cache_dir, exist_ok=True)
            norm = re.sub(rb"/[A-Za-z0-9_./-]*\.py", b"SRC.py", bir_json)
            key = hashlib.sha256(norm).hexdigest()[:32]
            hit = os.path.join(cache_dir, key + ".neff")
            if os.path.exists(hit):
                dst = os.path.join(tmpdir, neff_name)
                shutil.copyfile(hit, dst)
                return dst
            neff_path = orig(bir_json, tmpdir, neff_name=neff_name)
            tmp = tempfile.NamedTemporaryFile(
                dir=cache_dir, delete=False, suffix=".part"
            )
            tmp.close()
            shutil.copyfile(neff_path, tmp.name)
            os.replace(tmp.name, hit)
            return neff_path
        except Exception:
            return orig(bir_json, tmpdir, neff_name=neff_name)

    bass2jax.compile_bir_kernel = cached
    bass2jax._att_neff_cache = True


def _get_exec(**build_kwargs):
    """Build + jit-compile the SPMD executable once per process."""
    key = tuple(sorted(build_kwargs.items()))
    if key in _CACHE:
        return _CACHE[key]

    import jax
    from jax.sharding import Mesh, NamedSharding, PartitionSpec
    from jax.experimental.shard_map import shard_map

    from concourse import bass2jax

    bass2jax.install_neuronx_cc_hook()
    _enable_neff_cache(bass2jax)
    nc = build_bass(**build_kwargs)

    partition_name = nc.partition_id_tensor.name if nc.partition_id_tensor else None
    in_names, out_names, out_avals = [], [], []
    for alloc in nc.m.functions[0].allocations:
        if not isinstance(alloc, mybir.MemoryLocationSet):
            continue
        name = alloc.memorylocations[0].name
        if alloc.kind == "ExternalInput":
            if name != partition_name:
                in_names.append(name)
        elif alloc.kind == "ExternalOutput":
            out_names.append(name)
            out_avals.append(
                jax.core.ShapedArray(
                    tuple(alloc.tensor_shape), mybir.dt.np(alloc.dtype)
                )
            )
    all_in_names = list(in_names) + list(out_names)
    if partition_name is not None:
        all_in_names.append(partition_name)

    def _body(*args_):
        operands = list(args_)
        if partition_name is not None:
            operands.append(bass2jax.partition_id_tensor())
        return tuple(
            bass2jax._bass_exec_p.bind(
                *operands,
                out_avals=tuple(out_avals),
                in_names=tuple(all_in_names),
                out_names=tuple(out_names),
                lowering_input_output_aliases=(),
                sim_require_finite=True,
                sim_require_nnan=True,
                nc=nc,
            )
        )

    devices = jax.devices()[:N_CORES]
    mesh = Mesh(np.asarray(devices), ("core",))
    nin = len(in_names) + len(out_names)
    fn = jax.jit(
        shard_map(
            _body,
            mesh=mesh,
            in_specs=(PartitionSpec("core"),) * nin,
            out_specs=(PartitionSpec("core"),) * len(out_avals),
            check_rep=False,
        ),
        keep_unused=True,
    )
    shard = NamedSharding(mesh, PartitionSpec("core"))
    zeros = [
        jax.device_put(
            np.zeros((N_CORES * av.shape[0], *av.shape[1:]), av.dtype), shard
        )
        for av in out_avals
    ]
    _CACHE[key] = (fn, in_names, zeros, shard)
    return _CACHE[key]


def kernel(
    true_batch_size=None,
    temporal_edge_feature=None,
    edge_feature_s=None,
    W=None,
    b=None,
    **_unused,
):
    T = np.ascontiguousarray(np.asarray(temporal_edge_feature, dtype=np.float32))
    edge = np.asarray(edge_feature_s, dtype=np.float32)
    W = np.asarray(W, dtype=np.float32)
    b = np.asarray(b, dtype=np.float32)
    assert T.shape == (B, D) and edge.shape == (E, B, D)

    fn, in_names, zeros, shard = _get_exec()
    feed = make_feed(T, edge, W, b)

    import jax

    dev_in = [jax.device_put(feed[n], shard) for n in in_names]
    (out_concat,) = fn(*dev_in, *zeros)
    out_bed = np.asarray(out_concat)  # [B, E, D] in the device out dtype
    return np.ascontiguousarray(out_bed.astype(np.float32).transpose(1, 0, 2))
